# revision 9
# baseline (speedup 1.0000x reference)
"""Trainium2 Bass kernel: ComplexGabor1D layer.

reference math (fp32):
    lin = x @ W.T + b                      # [N, 256]
    env = exp(-3600 * lin^2)
    out = stack([env*cos(30*lin), env*sin(30*lin)], -1)   # [N, 256, 2]

Strategy (8 NeuronCores, data parallel over N):
  * Host: transpose each x shard to [256, N_SH] bf16 so the contraction dim
    lands on SBUF partitions with contiguous DMA loads; replicate W.T (bf16)
    and the bias (pre-broadcast fp32). bf16 inputs halve the input HBM
    traffic and double PE matmul rate; the resulting |dlin| ~ 3e-5 is far
    inside the 2e-2 output tolerance.
  * Device, per 2048-row block: bf16 matmuls accumulate lin into PSUM fp32;
    a DVE scalar_tensor_tensor drains PSUM to a bf16 lin tile while adding
    the bias. ACT then runs exactly three passes per element:
      imag' = sin(30*lin)          (Sin table)
      real' = sin(30*lin + pi/2)   (= cos, same table)
      env'  = Derivative_Erf(60*lin) = 2/sqrt(pi) * exp(-3600*lin^2)
    Derivative_Erf IS the Gabor envelope up to the 2/sqrt(pi) factor, so no
    Square/Exp passes are needed. DVE folds sqrt(pi)/2 into env with a 4x
    tensor_scalar, then multiplies env into both planes with 2x bf16
    tensor_tensor ops. ACT is the bottleneck engine at ~85% busy; its three
    passes are the floor (no table set fuses trig with a gaussian, and DVE
    polynomial substitutes cost ~3x what they save).
  * Output is written PLANAR bf16 ([block, p, half, plane, chunk, out], one
    DMA per plane with 4 KiB runs); the host de-interleaves and upcasts to
    fp32. bf16 output rounding (~2e-3) is well inside tolerance.
  * sin and derivative_erf live in different ACT table sets (~2.6us per
    switch = load + pipeline drain), so blocks are processed in groups
    ([6,5,5] for 16 blocks): all trig for a group, then all envelope -> 2
    switches per group, 6 loads total. A dummy sin at program start pulls
    the first table load into the pipeline-fill window. The ACT instruction
    order is pinned via dep edges.
  * The matmul+drain work of group g+1 is software-pipelined: its first
    block is emitted between trig(g) and env(g), the rest interleaved into
    env(g), so the in-order DVE stream issues the next group's PSUM drains
    before/between this group's envelope multiplies and the ACT never waits
    on a drain at a group boundary. Block 0's trig is emitted per half so
    the first sin starts after half a block's worth of DMA+matmul+drain.
  * sin table is accurate to |x| ~ 4 (measured); our max |arg| is ~3.3 and
    the envelope there is < 1e-8, so no range reduction is needed.
"""

import math

import numpy as np
from ml_dtypes import bfloat16

import concourse.bacc as bacc
import concourse.mybir as mybir
import concourse.tile as tile
from concourse.bass_utils import run_bass_kernel_spmd

N_TOTAL = 262144
IN_F = 256
OUT_F = 256
N_CORES = 8
N_SH = N_TOTAL // N_CORES  # 32768 rows per core

CHUNK = 128    # rows per matmul (PSUM partition dim)
HALF = 1024    # rows per PSUM tile (8 chunks)
BLOCK = 2048   # rows per ACT/DVE superblock (FD=4096 per instruction)
N_GROUPS = 3   # ACT-table-set groups (2 table switches per group)

OMEGA = 30.0
DERF_SCALE = 60.0           # Derivative_Erf(60*lin) = 2/sqrt(pi)*exp(-3600*lin^2)
SQRTPI_2 = math.sqrt(math.pi) / 2

F32 = mybir.dt.float32
BF16 = mybir.dt.bfloat16

_BUILD_CACHE = {}


def _build(n_sh, n_groups):
    """Build the single-core Bass program (SPMD across cores via in_maps)."""
    key = (n_sh, n_groups)
    if key in _BUILD_CACHE:
        return _BUILD_CACHE[key]

    assert n_sh % BLOCK == 0
    n_blocks = n_sh // BLOCK
    cph = HALF // CHUNK  # chunks per PSUM tile (8)

    # group sizes, as equal as possible, larger first: e.g. 16 -> [6, 5, 5]
    base, rem = divmod(n_blocks, n_groups)
    sizes = [base + (1 if i < rem else 0) for i in range(n_groups)]
    groups, pos = [], 0
    for sz in sizes:
        groups.append(list(range(pos, pos + sz)))
        pos += sz

    nc = bacc.Bacc("TRN2", target_bir_lowering=False, debug=False)

    xt = nc.dram_tensor("xt", [IN_F, n_sh], BF16, kind="ExternalInput").ap()
    wt = nc.dram_tensor("wt", [IN_F, OUT_F], BF16, kind="ExternalInput").ap()
    bias = nc.dram_tensor("bias", [CHUNK, cph * OUT_F], F32, kind="ExternalInput").ap()
    # row n = blk*2048 + h*1024 + c*128 + p ; plane e in {real, imag}
    out = nc.dram_tensor(
        "out", [n_blocks, CHUNK, 2, 2, cph, OUT_F], BF16, kind="ExternalOutput"
    ).ap()

    # [i, n] -> [p, ci, n] with i = ci*128 + p
    xt_r = xt.rearrange("(ci p) n -> p ci n", p=CHUNK)
    wt_r = wt.rearrange("(ci p) o -> p ci o", p=CHUNK)

    with tile.TileContext(nc) as tc:
        with (
            tc.tile_pool(name="consts", bufs=1) as consts,
            tc.tile_pool(name="xt", bufs=4) as xt_pool,
            tc.tile_pool(name="lin", bufs=10) as lin_pool,
            tc.tile_pool(name="outp", bufs=6) as out_pool,
            tc.tile_pool(name="ps", bufs=2, space="PSUM") as psum_pool,
        ):
            # consts go through the SWDGE queue so they don't head-of-line
            # block the first xt loads on the sync queue
            wt_sb = consts.tile([CHUNK, 2, OUT_F], BF16)
            nc.gpsimd.dma_start(wt_sb[:], wt_r[:])
            b_sb = consts.tile([CHUNK, cph, OUT_F], F32)
            nc.gpsimd.dma_start(b_sb[:], bias.rearrange("p (c o) -> p c o", c=cph))
            zero_b = consts.tile([CHUNK, 1], F32)
            nc.vector.memset(zero_b[:], 0.0)
            pio2_b = consts.tile([CHUNK, 1], F32)
            nc.vector.memset(pio2_b[:], math.pi / 2)

            prev_act = [None]

            def act_chain(inst):
                # Pin the ACT engine's instruction order to emission order so
                # the scheduler cannot interleave derivative_erf into the sin
                # stream (each jump costs two ~1.3us ACT table loads).
                if prev_act[0] is not None:
                    tile.add_dep_helper(inst.ins, prev_act[0], sync=False,
                                        reason="act table-set order")
                prev_act[0] = inst.ins

            # dummy sin: pulls the first Sin table load into the pipeline-fill
            # window so the first real trig instruction doesn't pay it
            warm = consts.tile([CHUNK, 1], BF16)
            act_chain(nc.scalar.activation(
                warm[:], zero_b[:], mybir.ActivationFunctionType.Sin,
                bias=zero_b[:], scale=OMEGA,
            ))

            lin_tiles = {}
            out_tiles = {}

            def phase_a(blk):
                # per half: load xt, matmul into PSUM, drain+bias to bf16 SBUF
                lin_sb = lin_pool.tile([CHUNK, 2, cph, OUT_F], BF16, tag="lin")
                for h in range(2):
                    n0 = blk * BLOCK + h * HALF
                    xt_t = xt_pool.tile([CHUNK, 2, HALF], BF16)
                    nc.sync.dma_start(xt_t[:], xt_r[:, :, n0 : n0 + HALF])
                    ps = psum_pool.tile([CHUNK, cph, OUT_F], F32)
                    for c in range(cph):
                        r0 = c * CHUNK
                        for ci in range(2):
                            nc.tensor.matmul(
                                ps[:, c, :],
                                xt_t[:, ci, r0 : r0 + CHUNK],
                                wt_sb[:, ci, :],
                                start=(ci == 0),
                                stop=(ci == 1),
                            )
                    # drain PSUM with a fused bias add: lin_sb = lin + b (bf16)
                    nc.vector.scalar_tensor_tensor(
                        lin_sb[:, h],
                        ps[:],
                        1.0,
                        b_sb[:],
                        op0=mybir.AluOpType.mult,
                        op1=mybir.AluOpType.add,
                    )
                lin_tiles[blk] = lin_sb

            def trig(blk, per_half):
                lin_sb = lin_tiles[blk]
                out_t = out_pool.tile([CHUNK, 2, 2, cph, OUT_F], BF16)
                out_tiles[blk] = out_t
                halves = [(h,) for h in range(2)] if per_half else [(slice(None),)]
                for (h,) in halves:
                    act_chain(nc.scalar.activation(
                        out_t[:, h, 1],
                        lin_sb[:, h],
                        mybir.ActivationFunctionType.Sin,
                        bias=zero_b[:],
                        scale=OMEGA,
                    ))
                    act_chain(nc.scalar.activation(
                        out_t[:, h, 0],
                        lin_sb[:, h],
                        mybir.ActivationFunctionType.Sin,
                        bias=pio2_b[:],
                        scale=OMEGA,
                    ))
                # fold sqrt(pi)/2 into the trig planes, on GPSIMD so the DVE
                # keeps its cycles for drains and envelope multiplies
                for e in range(2):
                    nc.gpsimd.tensor_scalar_mul(
                        out_t[:, :, e], out_t[:, :, e], SQRTPI_2
                    )

            def mul_store(blk, env_t):
                # multiply the envelope into both planes, store each plane
                out_t = out_tiles.pop(blk)
                for e in range(2):
                    nc.vector.tensor_mul(out_t[:, :, e], out_t[:, :, e], env_t[:])
                    # SWDGE so output stores don't head-of-line block loads;
                    # per plane so the store starts right after its multiply
                    nc.gpsimd.dma_start(out[blk][:, :, e], out_t[:, :, e])

            def env(blk, in_place):
                lin_sb = lin_tiles[blk]
                if in_place:
                    env_t = lin_sb
                    lin_tiles.pop(blk)
                else:
                    env_t = lin_pool.tile([CHUNK, 2, cph, OUT_F], BF16, tag="lin")
                act_chain(nc.scalar.activation(
                    env_t[:],
                    lin_sb[:],
                    mybir.ActivationFunctionType.Derivative_Erf,
                    bias=zero_b[:],
                    scale=DERF_SCALE,
                ))
                return env_t

            for blk in groups[0]:
                phase_a(blk)

            # Phase schedule: T0 E0 T1 E1 ... E_last T_last -- the last group
            # runs envelope-first (derf into separate tiles, adjacent to the
            # previous env phase: one fewer table switch) and its multiplies
            # pipeline under the final trig stream instead of trailing it.
            for gi, grp in enumerate(groups[:-1]):
                nxt = groups[gi + 1]
                last_nxt = gi + 1 == len(groups) - 1
                # ---- trig phase (sin table set resident) ----
                for k, blk in enumerate(grp):
                    trig(blk, per_half=(gi == 0 and k == 0))
                # next group's first block, emitted here so its PSUM drain
                # precedes this group's multiplies in the in-order DVE stream
                phase_a(nxt[0])
                # ---- envelope phase (erf_derivative table set resident) ----
                for j, blk in enumerate(grp):
                    env_t = env(blk, in_place=True)
                    mul_store(blk, env_t)
                    if j + 1 < len(nxt):
                        phase_a(nxt[j + 1])
                if last_nxt:
                    # stay on the erf_derivative set: last group's envelopes
                    env_ts = {blk: env(blk, in_place=False) for blk in nxt}
            # ---- final trig phase + multiplies ----
            for k, blk in enumerate(groups[-1]):
                trig(blk, per_half=False)
                env_t = env_ts.pop(blk)
                mul_store(blk, env_t)

    nc.compile()
    _BUILD_CACHE[key] = nc
    return nc


def run_sharded(x, W, b, trace=False, n_sh=N_SH, n_groups=N_GROUPS):
    """Shard inputs over the 8 cores, run the Bass kernel, gather output."""
    x = np.asarray(x, dtype=np.float32)
    W = np.asarray(W, dtype=np.float32)
    b = np.asarray(b, dtype=np.float32)
    n = x.shape[0]
    assert n == n_sh * N_CORES and x.shape[1] == IN_F

    nc = _build(n_sh, n_groups)

    cph = HALF // CHUNK
    wt_np = np.ascontiguousarray(W.T).astype(bfloat16)  # [in, out]
    b_np = np.ascontiguousarray(
        np.broadcast_to(np.tile(b, cph)[None, :], (CHUNK, cph * OUT_F))
    )
    in_maps = []
    for s in range(N_CORES):
        xt_np = np.ascontiguousarray(
            x[s * n_sh : (s + 1) * n_sh].T.astype(bfloat16)
        )  # [in, n_sh] bf16
        in_maps.append({"xt": xt_np, "wt": wt_np, "bias": b_np})

    res = run_bass_kernel_spmd(nc, in_maps, list(range(N_CORES)), trace=trace)

    n_blocks = n_sh // BLOCK
    shards = []
    for s in range(N_CORES):
        arr = np.asarray(res.results[s]["out"])  # [blk, p, h, e, c, o] bf16
        arr = arr.reshape(n_blocks, CHUNK, 2, 2, cph, OUT_F)
        # row n = blk*2048 + h*1024 + c*128 + p ; want [n, o, e] fp32
        full = arr.transpose(0, 2, 4, 1, 5, 3).reshape(n_sh, OUT_F, 2)
        shards.append(full.astype(np.float32))
    return np.concatenate(shards, axis=0), res


def kernel(x, W, b):
    out, _ = run_sharded(x, W, b)
    return out


# revision 10
# speedup vs baseline: 9.2165x; 9.2165x over previous
"""Trainium2 Bass kernel: ComplexGabor1D layer.

reference math (fp32):
    lin = x @ W.T + b                      # [N, 256]
    env = exp(-3600 * lin^2)
    out = stack([env*cos(30*lin), env*sin(30*lin)], -1)   # [N, 256, 2]

Strategy (8 NeuronCores, data parallel over N):
  * Host: transpose each x shard to [256, N_SH] bf16 so the contraction dim
    lands on SBUF partitions with contiguous DMA loads; replicate W.T (bf16)
    and the bias (pre-broadcast fp32). bf16 inputs halve the input HBM
    traffic and double PE matmul rate; the resulting |dlin| ~ 3e-5 is far
    inside the 2e-2 output tolerance.
  * Device, per 2048-row block: bf16 matmuls accumulate lin into PSUM fp32;
    a DVE scalar_tensor_tensor drains PSUM to a bf16 lin tile while adding
    the bias. ACT then runs exactly three passes per element:
      imag' = sin(30*lin)          (Sin table)
      real' = sin(30*lin + pi/2)   (= cos, same table)
      env'  = Derivative_Erf(60*lin) = 2/sqrt(pi) * exp(-3600*lin^2)
    Derivative_Erf IS the Gabor envelope up to the 2/sqrt(pi) factor, so no
    Square/Exp passes are needed. DVE folds sqrt(pi)/2 into env with a 4x
    tensor_scalar, then multiplies env into both planes with 2x bf16
    tensor_tensor ops. ACT is the bottleneck engine at ~85% busy; its three
    passes are the floor (no table set fuses trig with a gaussian, and DVE
    polynomial substitutes cost ~3x what they save).
  * Output is written PLANAR bf16 ([block, p, half, plane, chunk, out], one
    DMA per plane with 4 KiB runs); the host de-interleaves and upcasts to
    fp32. bf16 output rounding (~2e-3) is well inside tolerance.
  * sin and derivative_erf live in different ACT table sets (~2.6us per
    switch = load + pipeline drain), so blocks are processed in groups
    ([6,5,5] for 16 blocks): all trig for a group, then all envelope -> 2
    switches per group, 6 loads total. A dummy sin at program start pulls
    the first table load into the pipeline-fill window. The ACT instruction
    order is pinned via dep edges.
  * The matmul+drain work of group g+1 is software-pipelined: its first
    block is emitted between trig(g) and env(g), the rest interleaved into
    env(g), so the in-order DVE stream issues the next group's PSUM drains
    before/between this group's envelope multiplies and the ACT never waits
    on a drain at a group boundary. Block 0's trig is emitted per half so
    the first sin starts after half a block's worth of DMA+matmul+drain.
  * sin table is accurate to |x| ~ 4 (measured); our max |arg| is ~3.3 and
    the envelope there is < 1e-8, so no range reduction is needed.
"""

import math

import numpy as np
from ml_dtypes import bfloat16

import concourse.bacc as bacc
import concourse.mybir as mybir
import concourse.tile as tile
from concourse.bass_utils import run_bass_kernel_spmd

N_TOTAL = 262144
IN_F = 256
OUT_F = 256
N_CORES = 8
N_SH = N_TOTAL // N_CORES  # 32768 rows per core

CHUNK = 128    # rows per matmul (PSUM partition dim)
HALF = 1024    # rows per PSUM tile (8 chunks)
BLOCK = 2048   # rows per ACT/DVE superblock (FD=4096 per instruction)
N_GROUPS = 3   # ACT-table-set groups (2 table switches per group)

OMEGA = 30.0
DERF_SCALE = 60.0           # Derivative_Erf(60*lin) = 2/sqrt(pi)*exp(-3600*lin^2)
SQRTPI_2 = math.sqrt(math.pi) / 2

F32 = mybir.dt.float32
BF16 = mybir.dt.bfloat16

_BUILD_CACHE = {}


def _build(n_sh, n_groups):
    """Build the single-core Bass program (SPMD across cores via in_maps)."""
    key = (n_sh, n_groups)
    if key in _BUILD_CACHE:
        return _BUILD_CACHE[key]

    assert n_sh % BLOCK == 0
    n_blocks = n_sh // BLOCK
    cph = HALF // CHUNK  # chunks per PSUM tile (8)

    # group sizes, as equal as possible, larger first: e.g. 16 -> [6, 5, 5]
    base, rem = divmod(n_blocks, n_groups)
    sizes = [base + (1 if i < rem else 0) for i in range(n_groups)]
    groups, pos = [], 0
    for sz in sizes:
        groups.append(list(range(pos, pos + sz)))
        pos += sz

    nc = bacc.Bacc("TRN2", target_bir_lowering=False, debug=False)

    xt = nc.dram_tensor("xt", [IN_F, n_sh], BF16, kind="ExternalInput").ap()
    wt = nc.dram_tensor("wt", [IN_F, OUT_F], BF16, kind="ExternalInput").ap()
    bias = nc.dram_tensor("bias", [CHUNK, cph * OUT_F], F32, kind="ExternalInput").ap()
    # row n = blk*2048 + h*1024 + c*128 + p ; plane e in {real, imag}
    out = nc.dram_tensor(
        "out", [n_blocks, CHUNK, 2, 2, cph, OUT_F], BF16, kind="ExternalOutput"
    ).ap()

    # [i, n] -> [p, ci, n] with i = ci*128 + p
    xt_r = xt.rearrange("(ci p) n -> p ci n", p=CHUNK)
    wt_r = wt.rearrange("(ci p) o -> p ci o", p=CHUNK)

    with tile.TileContext(nc) as tc:
        with (
            tc.tile_pool(name="consts", bufs=1) as consts,
            tc.tile_pool(name="xt", bufs=4) as xt_pool,
            tc.tile_pool(name="lin", bufs=10) as lin_pool,
            tc.tile_pool(name="outp", bufs=6) as out_pool,
            tc.tile_pool(name="ps", bufs=2, space="PSUM") as psum_pool,
        ):
            # consts go through the SWDGE queue so they don't head-of-line
            # block the first xt loads on the sync queue
            wt_sb = consts.tile([CHUNK, 2, OUT_F], BF16)
            nc.gpsimd.dma_start(wt_sb[:], wt_r[:])
            b_sb = consts.tile([CHUNK, cph, OUT_F], F32)
            nc.gpsimd.dma_start(b_sb[:], bias.rearrange("p (c o) -> p c o", c=cph))
            zero_b = consts.tile([CHUNK, 1], F32)
            nc.vector.memset(zero_b[:], 0.0)
            pio2_b = consts.tile([CHUNK, 1], F32)
            nc.vector.memset(pio2_b[:], math.pi / 2)

            prev_act = [None]

            def act_chain(inst):
                # Pin the ACT engine's instruction order to emission order so
                # the scheduler cannot interleave derivative_erf into the sin
                # stream (each jump costs two ~1.3us ACT table loads).
                if prev_act[0] is not None:
                    tile.add_dep_helper(inst.ins, prev_act[0], sync=False,
                                        reason="act table-set order")
                prev_act[0] = inst.ins

            # dummy sin: pulls the first Sin table load into the pipeline-fill
            # window so the first real trig instruction doesn't pay it
            warm = consts.tile([CHUNK, 1], BF16)
            act_chain(nc.scalar.activation(
                warm[:], zero_b[:], mybir.ActivationFunctionType.Sin,
                bias=zero_b[:], scale=OMEGA,
            ))

            lin_tiles = {}
            out_tiles = {}

            def phase_a(blk):
                # per half: load xt, matmul into PSUM, drain+bias to bf16 SBUF
                lin_sb = lin_pool.tile([CHUNK, 2, cph, OUT_F], BF16, tag="lin")
                for h in range(2):
                    n0 = blk * BLOCK + h * HALF
                    xt_t = xt_pool.tile([CHUNK, 2, HALF], BF16)
                    nc.sync.dma_start(xt_t[:], xt_r[:, :, n0 : n0 + HALF])
                    ps = psum_pool.tile([CHUNK, cph, OUT_F], F32)
                    for c in range(cph):
                        r0 = c * CHUNK
                        for ci in range(2):
                            nc.tensor.matmul(
                                ps[:, c, :],
                                xt_t[:, ci, r0 : r0 + CHUNK],
                                wt_sb[:, ci, :],
                                start=(ci == 0),
                                stop=(ci == 1),
                            )
                    # drain PSUM with a fused bias add: lin_sb = lin + b (bf16)
                    nc.vector.scalar_tensor_tensor(
                        lin_sb[:, h],
                        ps[:],
                        1.0,
                        b_sb[:],
                        op0=mybir.AluOpType.mult,
                        op1=mybir.AluOpType.add,
                    )
                lin_tiles[blk] = lin_sb

            def trig(blk, per_half):
                lin_sb = lin_tiles[blk]
                out_t = out_pool.tile([CHUNK, 2, 2, cph, OUT_F], BF16)
                out_tiles[blk] = out_t
                halves = [(h,) for h in range(2)] if per_half else [(slice(None),)]
                for (h,) in halves:
                    act_chain(nc.scalar.activation(
                        out_t[:, h, 1],
                        lin_sb[:, h],
                        mybir.ActivationFunctionType.Sin,
                        bias=zero_b[:],
                        scale=OMEGA,
                    ))
                    act_chain(nc.scalar.activation(
                        out_t[:, h, 0],
                        lin_sb[:, h],
                        mybir.ActivationFunctionType.Sin,
                        bias=pio2_b[:],
                        scale=OMEGA,
                    ))

            def mul_store(blk, env_t):
                # multiply the envelope into both planes, store each plane
                out_t = out_tiles.pop(blk)
                for e in range(2):
                    nc.vector.tensor_mul(out_t[:, :, e], out_t[:, :, e], env_t[:])
                    # SWDGE so output stores don't head-of-line block loads;
                    # per plane so the store starts right after its multiply
                    nc.gpsimd.dma_start(out[blk][:, :, e], out_t[:, :, e])

            def env(blk, in_place):
                lin_sb = lin_tiles[blk]
                if in_place:
                    env_t = lin_sb
                    lin_tiles.pop(blk)
                else:
                    env_t = lin_pool.tile([CHUNK, 2, cph, OUT_F], BF16, tag="lin")
                act_chain(nc.scalar.activation(
                    env_t[:],
                    lin_sb[:],
                    mybir.ActivationFunctionType.Derivative_Erf,
                    bias=zero_b[:],
                    scale=DERF_SCALE,
                ))
                # fold sqrt(pi)/2 into the envelope (4x bf16 tensor_scalar)
                nc.vector.tensor_scalar_mul(env_t[:], env_t[:], SQRTPI_2)
                return env_t

            for blk in groups[0]:
                phase_a(blk)

            # Phase schedule: T0 E0 T1 E1 ... E_last T_last -- the last group
            # runs envelope-first (derf into separate tiles, adjacent to the
            # previous env phase: one fewer table switch) and its multiplies
            # pipeline under the final trig stream instead of trailing it.
            for gi, grp in enumerate(groups[:-1]):
                nxt = groups[gi + 1]
                last_nxt = gi + 1 == len(groups) - 1
                # ---- trig phase (sin table set resident) ----
                for k, blk in enumerate(grp):
                    trig(blk, per_half=(gi == 0 and k == 0))
                # next group's first block, emitted here so its PSUM drain
                # precedes this group's multiplies in the in-order DVE stream
                phase_a(nxt[0])
                # ---- envelope phase (erf_derivative table set resident) ----
                for j, blk in enumerate(grp):
                    env_t = env(blk, in_place=True)
                    mul_store(blk, env_t)
                    if j + 1 < len(nxt):
                        phase_a(nxt[j + 1])
                if last_nxt:
                    # stay on the erf_derivative set: last group's envelopes
                    env_ts = {blk: env(blk, in_place=False) for blk in nxt}
            # ---- final trig phase + multiplies ----
            for k, blk in enumerate(groups[-1]):
                trig(blk, per_half=False)
                env_t = env_ts.pop(blk)
                mul_store(blk, env_t)

    nc.compile()
    _BUILD_CACHE[key] = nc
    return nc


def run_sharded(x, W, b, trace=False, n_sh=N_SH, n_groups=N_GROUPS):
    """Shard inputs over the 8 cores, run the Bass kernel, gather output."""
    x = np.asarray(x, dtype=np.float32)
    W = np.asarray(W, dtype=np.float32)
    b = np.asarray(b, dtype=np.float32)
    n = x.shape[0]
    assert n == n_sh * N_CORES and x.shape[1] == IN_F

    nc = _build(n_sh, n_groups)

    cph = HALF // CHUNK
    wt_np = np.ascontiguousarray(W.T).astype(bfloat16)  # [in, out]
    b_np = np.ascontiguousarray(
        np.broadcast_to(np.tile(b, cph)[None, :], (CHUNK, cph * OUT_F))
    )
    in_maps = []
    for s in range(N_CORES):
        xt_np = np.ascontiguousarray(
            x[s * n_sh : (s + 1) * n_sh].T.astype(bfloat16)
        )  # [in, n_sh] bf16
        in_maps.append({"xt": xt_np, "wt": wt_np, "bias": b_np})

    res = run_bass_kernel_spmd(nc, in_maps, list(range(N_CORES)), trace=trace)

    n_blocks = n_sh // BLOCK
    shards = []
    for s in range(N_CORES):
        arr = np.asarray(res.results[s]["out"])  # [blk, p, h, e, c, o] bf16
        arr = arr.reshape(n_blocks, CHUNK, 2, 2, cph, OUT_F)
        # row n = blk*2048 + h*1024 + c*128 + p ; want [n, o, e] fp32
        full = arr.transpose(0, 2, 4, 1, 5, 3).reshape(n_sh, OUT_F, 2)
        shards.append(full.astype(np.float32))
    return np.concatenate(shards, axis=0), res


def kernel(x, W, b):
    out, _ = run_sharded(x, W, b)
    return out


# revision 11
# speedup vs baseline: 9.2439x; 1.0030x over previous
"""Trainium2 Bass kernel: ComplexGabor1D layer.

reference math (fp32):
    lin = x @ W.T + b                      # [N, 256]
    env = exp(-3600 * lin^2)
    out = stack([env*cos(30*lin), env*sin(30*lin)], -1)   # [N, 256, 2]

Strategy (8 NeuronCores, data parallel over N):
  * Host: transpose each x shard to [256, N_SH] bf16 so the contraction dim
    lands on SBUF partitions with contiguous DMA loads; replicate W.T (bf16)
    and the bias (pre-broadcast fp32). bf16 inputs halve the input HBM
    traffic and double PE matmul rate; the resulting |dlin| ~ 3e-5 is far
    inside the 2e-2 output tolerance.
  * Device, per 2048-row block: bf16 matmuls accumulate lin into PSUM fp32;
    a DVE scalar_tensor_tensor drains PSUM to a bf16 lin tile while adding
    the bias. ACT then runs exactly three passes per element:
      imag' = sin(30*lin)          (Sin table)
      real' = sin(30*lin + pi/2)   (= cos, same table)
      env'  = Derivative_Erf(60*lin) = 2/sqrt(pi) * exp(-3600*lin^2)
    Derivative_Erf IS the Gabor envelope up to the 2/sqrt(pi) factor, so no
    Square/Exp passes are needed. DVE folds sqrt(pi)/2 into env with a 4x
    tensor_scalar, then multiplies env into both planes with 2x bf16
    tensor_tensor ops. ACT is the bottleneck engine at ~85% busy; its three
    passes are the floor (no table set fuses trig with a gaussian, and DVE
    polynomial substitutes cost ~3x what they save).
  * Output is written PLANAR bf16 ([block, p, half, plane, chunk, out], one
    DMA per plane with 4 KiB runs); the host de-interleaves and upcasts to
    fp32. bf16 output rounding (~2e-3) is well inside tolerance.
  * sin and derivative_erf live in different ACT table sets (~2.6us per
    switch = load + pipeline drain), so blocks are processed in groups
    ([6,5,5] for 16 blocks): all trig for a group, then all envelope -> 2
    switches per group, 6 loads total. A dummy sin at program start pulls
    the first table load into the pipeline-fill window. The ACT instruction
    order is pinned via dep edges.
  * The matmul+drain work of group g+1 is software-pipelined: its first
    block is emitted between trig(g) and env(g), the rest interleaved into
    env(g), so the in-order DVE stream issues the next group's PSUM drains
    before/between this group's envelope multiplies and the ACT never waits
    on a drain at a group boundary. Block 0's trig is emitted per half so
    the first sin starts after half a block's worth of DMA+matmul+drain.
  * sin table is accurate to |x| ~ 4 (measured); our max |arg| is ~3.3 and
    the envelope there is < 1e-8, so no range reduction is needed.
"""

import math

import numpy as np
from ml_dtypes import bfloat16

import concourse.bacc as bacc
import concourse.mybir as mybir
import concourse.tile as tile
from concourse.bass_utils import run_bass_kernel_spmd

N_TOTAL = 262144
IN_F = 256
OUT_F = 256
N_CORES = 8
N_SH = N_TOTAL // N_CORES  # 32768 rows per core

CHUNK = 128    # rows per matmul (PSUM partition dim)
HALF = 1024    # rows per PSUM tile (8 chunks)
BLOCK = 2048   # rows per ACT/DVE superblock (FD=4096 per instruction)
N_GROUPS = 3   # ACT-table-set groups (2 table switches per group)

OMEGA = 30.0
DERF_SCALE = 60.0           # Derivative_Erf(60*lin) = 2/sqrt(pi)*exp(-3600*lin^2)
SQRTPI_2 = math.sqrt(math.pi) / 2

F32 = mybir.dt.float32
BF16 = mybir.dt.bfloat16

_BUILD_CACHE = {}


def _build(n_sh, n_groups):
    """Build the single-core Bass program (SPMD across cores via in_maps)."""
    key = (n_sh, n_groups)
    if key in _BUILD_CACHE:
        return _BUILD_CACHE[key]

    assert n_sh % BLOCK == 0
    n_blocks = n_sh // BLOCK
    cph = HALF // CHUNK  # chunks per PSUM tile (8)

    # group sizes, as equal as possible, larger first: e.g. 16 -> [6, 5, 5]
    base, rem = divmod(n_blocks, n_groups)
    sizes = [base + (1 if i < rem else 0) for i in range(n_groups)]
    groups, pos = [], 0
    for sz in sizes:
        groups.append(list(range(pos, pos + sz)))
        pos += sz

    nc = bacc.Bacc("TRN2", target_bir_lowering=False, debug=False)

    xt = nc.dram_tensor("xt", [IN_F, n_sh], BF16, kind="ExternalInput").ap()
    wt = nc.dram_tensor("wt", [IN_F, OUT_F], BF16, kind="ExternalInput").ap()
    bias = nc.dram_tensor("bias", [CHUNK, cph * OUT_F], F32, kind="ExternalInput").ap()
    # row n = blk*2048 + h*1024 + c*128 + p ; plane e in {real, imag}
    out = nc.dram_tensor(
        "out", [n_blocks, CHUNK, 2, 2, cph, OUT_F], BF16, kind="ExternalOutput"
    ).ap()

    # [i, n] -> [p, ci, n] with i = ci*128 + p
    xt_r = xt.rearrange("(ci p) n -> p ci n", p=CHUNK)
    wt_r = wt.rearrange("(ci p) o -> p ci o", p=CHUNK)

    with tile.TileContext(nc) as tc:
        with (
            tc.tile_pool(name="consts", bufs=1) as consts,
            tc.tile_pool(name="xt", bufs=4) as xt_pool,
            tc.tile_pool(name="lin", bufs=10) as lin_pool,
            tc.tile_pool(name="outp", bufs=6) as out_pool,
            tc.tile_pool(name="ps", bufs=2, space="PSUM") as psum_pool,
        ):
            # consts go through the SWDGE queue so they don't head-of-line
            # block the first xt loads on the sync queue
            wt_sb = consts.tile([CHUNK, 2, OUT_F], BF16)
            nc.gpsimd.dma_start(wt_sb[:], wt_r[:])
            b_sb = consts.tile([CHUNK, cph, OUT_F], F32)
            nc.gpsimd.dma_start(b_sb[:], bias.rearrange("p (c o) -> p c o", c=cph))
            zero_b = consts.tile([CHUNK, 1], F32)
            nc.vector.memset(zero_b[:], 0.0)
            pio2_b = consts.tile([CHUNK, 1], F32)
            nc.vector.memset(pio2_b[:], math.pi / 2)

            prev_act = [None]

            def act_chain(inst):
                # Pin the ACT engine's instruction order to emission order so
                # the scheduler cannot interleave derivative_erf into the sin
                # stream (each jump costs two ~1.3us ACT table loads).
                if prev_act[0] is not None:
                    tile.add_dep_helper(inst.ins, prev_act[0], sync=False,
                                        reason="act table-set order")
                prev_act[0] = inst.ins

            # dummy sin: pulls the first Sin table load into the pipeline-fill
            # window so the first real trig instruction doesn't pay it
            warm = consts.tile([CHUNK, 1], BF16)
            act_chain(nc.scalar.activation(
                warm[:], zero_b[:], mybir.ActivationFunctionType.Sin,
                bias=zero_b[:], scale=OMEGA,
            ))

            lin_tiles = {}
            out_tiles = {}

            def phase_a(blk):
                # per half: load xt, matmul into PSUM, drain+bias to bf16 SBUF
                lin_sb = lin_pool.tile([CHUNK, 2, cph, OUT_F], BF16, tag="lin")
                for h in range(2):
                    n0 = blk * BLOCK + h * HALF
                    xt_t = xt_pool.tile([CHUNK, 2, HALF], BF16)
                    nc.sync.dma_start(xt_t[:], xt_r[:, :, n0 : n0 + HALF])
                    ps = psum_pool.tile([CHUNK, cph, OUT_F], F32)
                    for c in range(cph):
                        r0 = c * CHUNK
                        for ci in range(2):
                            nc.tensor.matmul(
                                ps[:, c, :],
                                xt_t[:, ci, r0 : r0 + CHUNK],
                                wt_sb[:, ci, :],
                                start=(ci == 0),
                                stop=(ci == 1),
                            )
                    # drain PSUM with a fused bias add: lin_sb = lin + b (bf16)
                    nc.vector.scalar_tensor_tensor(
                        lin_sb[:, h],
                        ps[:],
                        1.0,
                        b_sb[:],
                        op0=mybir.AluOpType.mult,
                        op1=mybir.AluOpType.add,
                    )
                lin_tiles[blk] = lin_sb

            def trig(blk, per_half):
                lin_sb = lin_tiles[blk]
                out_t = out_pool.tile([CHUNK, 2, 2, cph, OUT_F], BF16)
                out_tiles[blk] = out_t
                halves = [(h,) for h in range(2)] if per_half else [(slice(None),)]
                for (h,) in halves:
                    act_chain(nc.scalar.activation(
                        out_t[:, h, 1],
                        lin_sb[:, h],
                        mybir.ActivationFunctionType.Sin,
                        bias=zero_b[:],
                        scale=OMEGA,
                    ))
                    act_chain(nc.scalar.activation(
                        out_t[:, h, 0],
                        lin_sb[:, h],
                        mybir.ActivationFunctionType.Sin,
                        bias=pio2_b[:],
                        scale=OMEGA,
                    ))

            def mul_store(blk, env_t):
                # multiply the envelope into both planes, store each plane
                out_t = out_tiles.pop(blk)
                for e in range(2):
                    nc.vector.tensor_mul(out_t[:, :, e], out_t[:, :, e], env_t[:])
                    # SWDGE so output stores don't head-of-line block loads;
                    # per plane so the store starts right after its multiply
                    nc.gpsimd.dma_start(out[blk][:, :, e], out_t[:, :, e])

            def env(blk, in_place):
                lin_sb = lin_tiles[blk]
                if in_place:
                    env_t = lin_sb
                    lin_tiles.pop(blk)
                else:
                    env_t = lin_pool.tile([CHUNK, 2, cph, OUT_F], BF16, tag="lin")
                act_chain(nc.scalar.activation(
                    env_t[:],
                    lin_sb[:],
                    mybir.ActivationFunctionType.Derivative_Erf,
                    bias=zero_b[:],
                    scale=DERF_SCALE,
                ))
                # fold sqrt(pi)/2 into the envelope (4x bf16 tensor_scalar)
                nc.vector.tensor_scalar_mul(env_t[:], env_t[:], SQRTPI_2)
                return env_t

            for blk in groups[0]:
                phase_a(blk)

            # Phase schedule: T0 E0 T1 E1 ... E_last T_last -- the last group
            # runs envelope-first (derf into separate tiles, adjacent to the
            # previous env phase: one fewer table switch) and its multiplies
            # pipeline under the final trig stream instead of trailing it.
            for gi, grp in enumerate(groups[:-1]):
                nxt = groups[gi + 1]
                last_nxt = gi + 1 == len(groups) - 1
                # ---- trig phase (sin table set resident) ----
                # The next group's loads/matmuls/drains are emitted here,
                # where the in-order DVE stream is otherwise idle, so the env
                # phase's multiplies never delay a drain (and the next trig
                # phase never waits on one).
                for k, blk in enumerate(grp):
                    trig(blk, per_half=(gi == 0 and k == 0))
                    if k < len(nxt):
                        phase_a(nxt[k])
                for blk in nxt[len(grp):]:
                    phase_a(blk)
                # ---- envelope phase (erf_derivative table set resident) ----
                for blk in grp:
                    env_t = env(blk, in_place=True)
                    mul_store(blk, env_t)
                if last_nxt:
                    # stay on the erf_derivative set: last group's envelopes
                    env_ts = {blk: env(blk, in_place=False) for blk in nxt}
            # ---- final trig phase + multiplies ----
            for k, blk in enumerate(groups[-1]):
                trig(blk, per_half=False)
                env_t = env_ts.pop(blk)
                mul_store(blk, env_t)

    nc.compile()
    _BUILD_CACHE[key] = nc
    return nc


def run_sharded(x, W, b, trace=False, n_sh=N_SH, n_groups=N_GROUPS):
    """Shard inputs over the 8 cores, run the Bass kernel, gather output."""
    x = np.asarray(x, dtype=np.float32)
    W = np.asarray(W, dtype=np.float32)
    b = np.asarray(b, dtype=np.float32)
    n = x.shape[0]
    assert n == n_sh * N_CORES and x.shape[1] == IN_F

    nc = _build(n_sh, n_groups)

    cph = HALF // CHUNK
    wt_np = np.ascontiguousarray(W.T).astype(bfloat16)  # [in, out]
    b_np = np.ascontiguousarray(
        np.broadcast_to(np.tile(b, cph)[None, :], (CHUNK, cph * OUT_F))
    )
    in_maps = []
    for s in range(N_CORES):
        xt_np = np.ascontiguousarray(
            x[s * n_sh : (s + 1) * n_sh].T.astype(bfloat16)
        )  # [in, n_sh] bf16
        in_maps.append({"xt": xt_np, "wt": wt_np, "bias": b_np})

    res = run_bass_kernel_spmd(nc, in_maps, list(range(N_CORES)), trace=trace)

    n_blocks = n_sh // BLOCK
    shards = []
    for s in range(N_CORES):
        arr = np.asarray(res.results[s]["out"])  # [blk, p, h, e, c, o] bf16
        arr = arr.reshape(n_blocks, CHUNK, 2, 2, cph, OUT_F)
        # row n = blk*2048 + h*1024 + c*128 + p ; want [n, o, e] fp32
        full = arr.transpose(0, 2, 4, 1, 5, 3).reshape(n_sh, OUT_F, 2)
        shards.append(full.astype(np.float32))
    return np.concatenate(shards, axis=0), res


def kernel(x, W, b):
    out, _ = run_sharded(x, W, b)
    return out
